# revision 32
# baseline (speedup 1.0000x reference)
"""Trainium2 Bass kernel for the NeuralRenderer depth/silhouette rasterizer.

Strategy
--------
Host (numpy, cheap O(B*V + B*F) work):
  * perspective-project vertices (identical arithmetic to the reference),
  * gather per-face screen-space vertices (forward winding only -- the
    fill_back reversed duplicate is mathematically identical for this
    z-buffer: barycentrics are divided by the signed area, so both windings
    rasterize the same pixels with the same depths),
  * convert each face to affine plane coefficients (A,B,C) for the three
    scaled barycentric planes w0,w1,w2 (sign tests only; scaled by 1e20)
    and the perspective-correct inverse depth plane inv = w0/za+w1/zb+w2/zc;
    then max_{valid faces} inv per pixel = max_f min(w0,w1,w2,inv) and
    zbuf = 1/maxinv (no per-face reciprocals or selects needed),
  * tile the image into 8x16-pixel tiles (=128 pixels, one partition
    block), cull faces per tile by bbox overlap and conservative-exact
    early-z bounds (a face whose tile-local min depth exceeds the best
    full-tile-covering face's max corner depth can never win the z-test;
    cuts ~806M reference face-pixel pairs to ~7M),
  * pack tiles as variable-size positions (faces padded to a quantum of 4)
    into 512-column PSUM banks, 4 banks per group; one shared cross-core
    schedule (positions dealt size-sorted round-robin over the 8 cores,
    position sizes unified by max) keeps the SPMD instruction stream
    identical while each core rasterizes its own tiles.

Device (8 NeuronCores, SPMD, one group pipelined per PSUM half):
  TensorE: one bf16 matmul per position: lhsT [6,128] = tile-local pixel
  coords (Xl,Yl,1,Xl,Yl,1) -- exactly representable in bf16 -- and rhs
  [6,4s] = hi/lo-split plane coefficients (Ahi..Clo), so every product is
  exact and f32 PSUM accumulation yields ~17-mantissa-bit plane values at
  bf16 speed (fp32 matmuls are 4x slower per column on trn2).  Output is
  quantity-planar per bank: [w1|w0|w2|inv] x 128 pair-columns.
  The whole coefficient table loads once (chunked DMAs) into a resident
  SBUF tile -- no per-group DMA.  ScalarE: one strided bf16-cast copy
  evacuates the three sign-planes (w1,w0,w2) to SBUF; their mins run on
  VectorE in 2x bf16 mode (sign-exact; output unchanged because a valid
  face's HUGE-scaled w-min always exceeds inv, so the final mixed-dtype
  min against the f32 inv plane still in PSUM returns the exact f32 inv).
  A segmented reduce_max over quantum-4 sub-segments -> [128, 128]/group.

Host epilogue: per-position max over its sub-segments -> per-pixel max inv,
zbuf = 1/maxinv (FAR where uncovered), exact reference depth
normalization, mask = coverage.
"""

import math

import numpy as np

F32 = np.float32
FAR = F32(10.0)
RAST = 256
HUGE = F32(1e20)
TH, TW = 8, 16          # pixel tile = 8 rows x 16 cols = 128 pixels
NTY, NTX = RAST // TH, RAST // TW
SLOT = 64               # faces per slot
SPG = 8                 # slots per group
N_CORES = 8

# pixel-center coordinate grid in [-1, 1]
_G = (F32(2.0) * (np.arange(RAST, dtype=np.float32) + F32(0.5)) / F32(RAST)
      - F32(1.0)).astype(np.float32)

_PROGRAM_CACHE: dict = {}


# ----------------------------------------------------------------- host math

def _project(vertices, K, R, t, dist):
    """Match reference._project arithmetic in numpy float32."""
    EPS = F32(1e-9)
    v = np.einsum('bvj,bij->bvi', vertices, R).astype(np.float32) + t
    x, y, z = v[..., 0], v[..., 1], v[..., 2]
    x_ = x / (z + EPS)
    y_ = y / (z + EPS)
    k1, k2, p1, p2, k3 = (dist[:, None, i] for i in range(5))
    r2 = x_ * x_ + y_ * y_
    radial = F32(1.0) + k1 * r2 + k2 * r2 ** 2 + k3 * r2 ** 3
    x__ = x_ * radial + F32(2.0) * p1 * x_ * y_ + p2 * (r2 + F32(2.0) * x_ ** 2)
    y__ = y_ * radial + p1 * (r2 + F32(2.0) * y_ ** 2) + F32(2.0) * p2 * x_ * y_
    pts = np.stack([x__, y__, np.ones_like(z)], axis=-1).astype(np.float32)
    pts = np.einsum('bvj,bij->bvi', pts, K).astype(np.float32)
    u, vv = pts[..., 0], pts[..., 1]
    vv = F32(256.0) - vv
    u = F32(2.0) * (u - F32(128.0)) / F32(256.0)
    vv = F32(2.0) * (vv - F32(128.0)) / F32(256.0)
    return np.stack([u, vv, z], axis=-1).astype(np.float32)


def _face_coeffs(fv):
    """fv: [F,3,3] per-face (u,v,z).  Returns A,B,C [4,F] f32 plane coeffs
    (w0s,w1s,w2s scaled by HUGE; inv unscaled), plus validf, bbox, zmin."""
    ax, ay, az = fv[:, 0, 0], fv[:, 0, 1], fv[:, 0, 2]
    bx, by, bz = fv[:, 1, 0], fv[:, 1, 1], fv[:, 1, 2]
    cx, cy, cz = fv[:, 2, 0], fv[:, 2, 1], fv[:, 2, 2]
    den = (bx - ax) * (cy - ay) - (by - ay) * (cx - ax)
    ok = np.abs(den) > F32(1e-8)
    den_s = np.where(ok, den, F32(1.0)).astype(np.float32)
    a0 = (by - cy) / den_s
    b0 = (cx - bx) / den_s
    c0 = (-(by - cy) * cx - (cx - bx) * cy) / den_s
    a1 = (cy - ay) / den_s
    b1 = (ax - cx) / den_s
    c1 = (-(cy - ay) * cx - (ax - cx) * cy) / den_s
    a2 = -(a0 + a1)
    b2 = -(b0 + b1)
    c2 = F32(1.0) - (c0 + c1)
    zok = (az > F32(1e-8)) & (bz > F32(1e-8)) & (cz > F32(1e-8))
    ra = F32(1.0) / np.where(az > F32(1e-8), az, F32(1.0)).astype(np.float32)
    rb = F32(1.0) / np.where(bz > F32(1e-8), bz, F32(1.0)).astype(np.float32)
    rc = F32(1.0) / np.where(cz > F32(1e-8), cz, F32(1.0)).astype(np.float32)
    ai = a0 * ra + a1 * rb + a2 * rc
    bi = b0 * ra + b1 * rb + b2 * rc
    ci = c0 * ra + c1 * rb + c2 * rc
    validf = ok & zok
    A = np.stack([a0 * HUGE, a1 * HUGE, a2 * HUGE, ai]).astype(np.float32)
    B = np.stack([b0 * HUGE, b1 * HUGE, b2 * HUGE, bi]).astype(np.float32)
    C = np.stack([c0 * HUGE, c1 * HUGE, c2 * HUGE, ci]).astype(np.float32)
    # invalid faces: force w0 plane to -HUGE so they never pass the sign test
    C[0, ~validf] = -HUGE
    A[0, ~validf] = F32(0.0)
    B[0, ~validf] = F32(0.0)
    u = fv[:, :, 0]
    v = fv[:, :, 1]
    z = fv[:, :, 2]
    bbox = (u.min(1), u.max(1), v.min(1), v.max(1))
    return A, B, C, validf, bbox, z.min(1)


def _cull_tiles(A, B, C, validf, bbox, zmin):
    """Per-tile face lists with bbox + conservative early-z culling.
    Returns list of length NTY*NTX of int arrays."""
    umin, umax, vmin, vmax = bbox
    eps = F32(2.0 / 256.0)
    # tile pixel-center extents
    x0 = _G[np.arange(NTX) * TW]
    x1 = _G[np.arange(NTX) * TW + TW - 1]
    y0 = _G[np.arange(NTY) * TH]
    y1 = _G[np.arange(NTY) * TH + TH - 1]

    lists = []
    Fn = A.shape[1]
    for ty in range(NTY):
        # corner coords for the 16 tiles of this row: [NTX, 4]
        cxs = np.stack([x0, x1, x0, x1], axis=1).astype(np.float32)
        cys = np.stack(
            [np.full(NTX, y0[ty]), np.full(NTX, y0[ty]),
             np.full(NTX, y1[ty]), np.full(NTX, y1[ty])], axis=1
        ).astype(np.float32)
        # plane evals at corners: [4(quant), NTX, F, 4(corner)]
        q = (A[:, None, :, None] * cxs[None, :, None, :]
             + B[:, None, :, None] * cys[None, :, None, :]
             + C[:, None, :, None])
        wmin = np.minimum(np.minimum(q[0], q[1]), q[2]).min(-1)  # [NTX, F]
        cover = (wmin >= F32(1e-6) * HUGE) & validf[None, :]
        min_inv = q[3].min(-1)  # [NTX, F]
        pos = cover & (min_inv > F32(1e-7))
        zc = np.where(pos, F32(1.0) / np.where(pos, min_inv, F32(1.0)), np.inf)
        Zt = zc.min(1) * F32(1.001) + F32(1e-3)  # [NTX]
        ov = (validf[None, :]
              & (umax[None, :] >= x0[:, None] - eps)
              & (umin[None, :] <= x1[:, None] + eps)
              & (vmax[None, :] >= y0[ty] - eps)
              & (vmin[None, :] <= y1[ty] + eps))
        # tile-local lower bound on this face's depth: inv is affine, so its
        # max over the tile rect is attained at a corner; z >= 1/max_inv.
        max_inv = q[3].max(-1)  # [NTX, F]
        ov &= max_inv > F32(-1e-5)   # inv <= 0 on whole tile -> never valid
        zloc = np.where(max_inv > F32(1e-3),
                        (F32(1.0) / np.where(max_inv > F32(1e-3), max_inv,
                                             F32(1.0))) * F32(0.999) - F32(1e-3),
                        F32(0.0))
        zloc = np.maximum(zloc, zmin[None, :])
        sel = ov & (zloc <= Zt[:, None])
        for tx in range(NTX):
            lists.append(np.nonzero(sel[tx])[0])
    return lists


def _bf16_split(x):
    """Split f32 array into bf16 hi + lo parts (hi+lo ~ 16-17 mantissa bits)."""
    import ml_dtypes
    bf = ml_dtypes.bfloat16
    hi = x.astype(bf)
    lo = (x - hi.astype(np.float32)).astype(bf)
    return hi, lo


def _build_plan(vertices, faces, intr, R, t, dist_coeffs):
    """All host preprocessing; returns per-core tables + slot maps + G.

    Device table layout (bf16), per group g: [6, 3072]
      cols 0:2048   coeff blocks: 8 slots x [w1|w0|w2|inv] x 64 faces;
                    rows 0-2 = (Ahi,Bhi,Chi), rows 3-5 = (Alo,Blo,Clo)
      cols 2048:3072 coords: 8 slots x 128 px; rows = (Xl,Yl,1,Xl,Yl,1)
                    with tile-local Xl,Yl (exact in bf16).
    Plane constant C' is translated to the tile origin in float64.
    """
    Bn = vertices.shape[0]
    uvz = _project(vertices, intr, R, t, dist_coeffs)
    ar = np.arange(Bn)[:, None, None]
    fv = uvz[ar, faces]  # [B,F,3,3] forward winding only

    slots_by_core = [[] for _ in range(N_CORES)]
    coeffs_by_img = []
    all_slots = []
    for b in range(Bn):
        A, B, C, validf, bbox, zmin = _face_coeffs(fv[b])
        coeffs_by_img.append((A, B, C))
        lists = _cull_tiles(A, B, C, validf, bbox, zmin)
        for tidx, idxs in enumerate(lists):
            ty, tx = divmod(tidx, NTX)
            for s0 in range(0, max(len(idxs), 1), SLOT):
                all_slots.append((b, ty, tx, idxs[s0:s0 + SLOT]))
    # round-robin over all cores (results are merged on host, so any core
    # can own any slot; this balances the per-core group count)
    for i, sl in enumerate(all_slots):
        slots_by_core[i % N_CORES].append(sl)

    S = max(len(s) for s in slots_by_core)
    G = max(1, math.ceil(S / SPG))
    S = G * SPG

    import ml_dtypes
    bf = ml_dtypes.bfloat16
    QORD = (1, 0, 2, 3)  # psum quantity order per slot: (w1, w0, w2, inv)
    tabs = []
    for c in range(N_CORES):
        tab = np.zeros((G, 6, 3072), dtype=bf)
        tab[:, 2, 2048:] = 1.0  # coords row 2 = 1 (row 5 stays 0: lo half)
        # default: w0 C block = -HUGE for dummy slots / pad faces
        for s in range(SPG):
            tab[:, 2, s * 256 + 64:s * 256 + 128] = np.array(-HUGE, dtype=bf)
        for j, (b, ty, tx, idxs) in enumerate(slots_by_core[c]):
            g, s = divmod(j, SPG)
            A, B, C = coeffs_by_img[b]
            k = len(idxs)
            base = s * 256
            x0 = np.float64(_G[tx * TW])
            y0 = np.float64(_G[ty * TH])
            Af = A[:, idxs]
            Bf = B[:, idxs]
            Cp = (Af.astype(np.float64) * x0 + Bf.astype(np.float64) * y0
                  + C[:, idxs].astype(np.float64)).astype(np.float32)
            Ahi, Alo = _bf16_split(Af)
            Bhi, Blo = _bf16_split(Bf)
            Chi, Clo = _bf16_split(Cp)
            for qi, q in enumerate(QORD):
                o = base + qi * 64
                tab[g, 0, o:o + k] = Ahi[q]
                tab[g, 1, o:o + k] = Bhi[q]
                tab[g, 2, o:o + k] = Chi[q]
                tab[g, 3, o:o + k] = Alo[q]
                tab[g, 4, o:o + k] = Blo[q]
                tab[g, 5, o:o + k] = Clo[q]
            if k < SLOT:  # pad faces: w0 plane = -HUGE
                o = base + 1 * 64  # w0 is second in QORD
                tab[g, 0:6, o + k:o + 64] = 0
                tab[g, 2, o + k:o + 64] = np.array(-HUGE, dtype=bf)
            cb = 2048 + s * 128
            xs = (_G[tx * TW:(tx + 1) * TW] - _G[tx * TW]).astype(bf)
            ys = (_G[ty * TH:(ty + 1) * TH] - _G[ty * TH]).astype(bf)
            tab[g, 0, cb:cb + 128] = np.tile(xs, TH)
            tab[g, 1, cb:cb + 128] = np.repeat(ys, TW)
            tab[g, 3, cb:cb + 128] = tab[g, 0, cb:cb + 128]
            tab[g, 4, cb:cb + 128] = tab[g, 1, cb:cb + 128]
            tab[g, 5, cb:cb + 128] = 1.0
            # row 2 already 1.0
        tabs.append(tab)
    return tabs, slots_by_core, G


# ---------------------------------------------------- v2 plan: quantum-8 packing

Q = 4            # face quantum (pair-columns granularity)
SUB = 512 // Q   # reduce sub-results per group
BANKC = 128      # pair-columns per PSUM bank (512 f32 / 4 planes)


def _build_plan2(vertices, faces, intr, R, t, dist_coeffs):
    """Quantum-8 variable-size packing.  Returns (coef_tabs, crd_tabs, sched,
    G, Pmax, core_positions) where sched is the shared cross-core schedule:
    a list of groups; each group is a list of (size, bank_in_group, r).
    PSUM per bank is quantity-planar: [w1|w0|w2|inv] x 128 pair-cols."""
    import ml_dtypes
    bf = ml_dtypes.bfloat16
    Bn = vertices.shape[0]
    uvz = _project(vertices, intr, R, t, dist_coeffs)
    ar = np.arange(Bn)[:, None, None]
    fv = uvz[ar, faces]

    coeffs_by_img = []
    all_pos = []  # (b, ty, tx, idxs)
    for b in range(Bn):
        A, B, C, validf, bbox, zmin = _face_coeffs(fv[b])
        coeffs_by_img.append((A, B, C))
        lists = _cull_tiles(A, B, C, validf, bbox, zmin)
        for tidx, idxs in enumerate(lists):
            if len(idxs) == 0:
                continue
            ty, tx = divmod(tidx, NTX)
            for s0 in range(0, len(idxs), BANKC):
                all_pos.append((b, ty, tx, idxs[s0:s0 + BANKC]))

    sizes = np.array([math.ceil(len(p[3]) / Q) * Q for p in all_pos])
    order = np.argsort(-sizes, kind="stable")
    core_positions = [[] for _ in range(N_CORES)]
    for rank, oi in enumerate(order):
        core_positions[rank % N_CORES].append(all_pos[oi])
    ncnt = max(len(cp) for cp in core_positions)
    uni = np.zeros(ncnt, dtype=np.int64)
    for cp in core_positions:
        for i, p in enumerate(cp):
            uni[i] = max(uni[i], math.ceil(len(p[3]) / Q) * Q)

    # shared schedule: pack unified sizes into banks, dummy-fill remainders
    entries = []  # (size, real_index or None)
    cur = 0
    for i in range(ncnt):
        s = int(uni[i])
        if s == 0:
            continue
        if cur + s > BANKC:
            entries.append((BANKC - cur, None))
            cur = 0
        entries.append((s, i))
        cur = (cur + s) % BANKC
    if cur > 0:
        entries.append((BANKC - cur, None))
    # count banks, pad to multiple of 4
    nbank = sum(s for s, _ in entries) // BANKC
    while nbank % 4 != 0:
        entries.append((BANKC, None))
        nbank += 1
    G = nbank // 4

    # annotate entries with (group, bank_in_group, r, idx_in_group)
    sched = [[] for _ in range(G)]
    bank = 0
    r = 0
    for s, ridx in entries:
        g, big = divmod(bank, 4)
        sched[g].append((s, big, r, ridx))
        r += s
        if r == BANKC:
            bank += 1
            r = 0
    Pmax = max(len(grp) for grp in sched)

    # flat table: per group [coef 2048 cols | crd npos*128 cols]
    goff = []
    o = 0
    for grp in sched:
        goff.append(o)
        o += 2048 + len(grp) * 128
    tot_cols = o

    QORD = (1, 0, 2, 3)  # plane order in bank: (w1, w0, w2, inv)
    tabs = []
    for c in range(N_CORES):
        tab = np.zeros((6, tot_cols), dtype=bf)
        for g, grp in enumerate(sched):
            tab[2, goff[g] + 2048: goff[g] + 2048 + len(grp) * 128] = 1.0
            for j, (s, big, r, ridx) in enumerate(grp):
                co = goff[g] + big * 512 + 4 * r   # coeff col base: 4*s cols
                cb = goff[g] + 2048 + j * 128
                pos = None
                if ridx is not None and ridx < len(core_positions[c]):
                    pos = core_positions[c][ridx]
                if pos is None:
                    # dummy: w0 plane C = -HUGE (plane index 1 in QORD)
                    tab[2, co + s: co + 2 * s] = np.array(-HUGE, dtype=bf)
                    continue
                b, ty, tx, idxs = pos
                A, B, C = coeffs_by_img[b]
                k = len(idxs)
                x0 = np.float64(_G[tx * TW])
                y0 = np.float64(_G[ty * TH])
                Af = A[:, idxs]
                Bf = B[:, idxs]
                Cp = (Af.astype(np.float64) * x0 + Bf.astype(np.float64) * y0
                      + C[:, idxs].astype(np.float64)).astype(np.float32)
                Ahi, Alo = _bf16_split(Af)
                Bhi, Blo = _bf16_split(Bf)
                Chi, Clo = _bf16_split(Cp)
                for qi, q in enumerate(QORD):
                    o2 = co + qi * s
                    tab[0, o2:o2 + k] = Ahi[q]
                    tab[1, o2:o2 + k] = Bhi[q]
                    tab[2, o2:o2 + k] = Chi[q]
                    tab[3, o2:o2 + k] = Alo[q]
                    tab[4, o2:o2 + k] = Blo[q]
                    tab[5, o2:o2 + k] = Clo[q]
                if k < s:  # pad faces: w0 plane = -HUGE
                    o2 = co + 1 * s
                    tab[2, o2 + k:o2 + s] = np.array(-HUGE, dtype=bf)
                xs = (_G[tx * TW:(tx + 1) * TW] - _G[tx * TW]).astype(bf)
                ys = (_G[ty * TH:(ty + 1) * TH] - _G[ty * TH]).astype(bf)
                tab[0, cb:cb + 128] = np.tile(xs, TH)
                tab[1, cb:cb + 128] = np.repeat(ys, TW)
                tab[3, cb:cb + 128] = tab[0, cb:cb + 128]
                tab[4, cb:cb + 128] = tab[1, cb:cb + 128]
                tab[5, cb:cb + 128] = 1.0
        tabs.append(tab)
    return tabs, sched, G, goff, core_positions


def _build_program2(G, sched, goff, tot_cols):
    import concourse.mybir as mybir
    from concourse import bacc
    from concourse.tile import TileContext

    nc = bacc.Bacc(None, target_bir_lowering=False, debug=False)
    tab = nc.declare_dram_parameter("tab", [6, tot_cols], mybir.dt.bfloat16,
                                    isOutput=False)
    res = nc.declare_dram_parameter("res", [128, G * SUB], mybir.dt.float32,
                                    isOutput=True)
    mn = mybir.AluOpType.min
    mx = mybir.AluOpType.max
    f32 = mybir.dt.float32
    bf16 = mybir.dt.bfloat16

    # chunk the one-time table load at group boundaries so early groups start
    # while later chunks stream
    nchunk = min(16, G)
    cuts = [goff[(G * i) // nchunk] for i in range(nchunk)] + [tot_cols]

    with TileContext(nc) as tc:
        with (
            tc.tile_pool(name="tabs", bufs=1) as tab_pool,
            tc.tile_pool(name="work", bufs=3) as work_pool,
            tc.tile_pool(name="out", bufs=1) as out_pool,
            tc.tile_pool(name="psum", bufs=2, space="PSUM") as psum_pool,
        ):
            res_sb = out_pool.tile([128, G * SUB], f32)
            tabt = tab_pool.tile([6, tot_cols], bf16)
            for i in range(nchunk):
                nc.sync.dma_start(out=tabt[:, cuts[i]:cuts[i + 1]],
                                  in_=tab[:, cuts[i]:cuts[i + 1]])
            for g in range(G):
                grp = sched[g]
                base = goff[g]
                cb = base + 2048
                pt = psum_pool.tile([128, 2048], f32)
                ptb = pt[:].rearrange("p (b q c) -> p b q c", q=4, c=BANKC)
                for j, (s, big, r, _ridx) in enumerate(grp):
                    nc.tensor.matmul(
                        ptb[:, big, :, r:r + s],
                        lhsT=tabt[:, cb + j * 128: cb + (j + 1) * 128],
                        rhs=tabt[:, base + big * 512 + 4 * r:
                                 base + big * 512 + 4 * r + 4 * s],
                        start=True, stop=True,
                    )
                # planes per bank: 0=w1, 1=w0, 2=w2 (sign tests only ->
                # bf16-cast copy, mins run in 2x DVE mode), 3=inv (stays f32
                # in PSUM; the final min picks the exact f32 inv whenever the
                # face is valid since scaled w-mins are >> inv)
                c = work_pool.tile([128, 1536], bf16)
                cvT = c[:].rearrange("p (q b k) -> p b q k", q=3, k=BANKC)
                nc.scalar.copy(cvT[:, :, :, :], ptb[:, :, 0:3, :])
                mb1 = work_pool.tile([128, 512], bf16)
                mb2 = work_pool.tile([128, 512], bf16)
                tt = work_pool.tile([128, 512], f32)
                nc.vector.tensor_tensor(mb1[:], c[:, 0:512], c[:, 512:1024], mn)
                nc.vector.tensor_tensor(mb2[:], mb1[:], c[:, 1024:1536], mn)
                ttv = tt[:].rearrange("p (b k) -> p b k", k=BANKC)
                mb2v = mb2[:].rearrange("p (b k) -> p b k", k=BANKC)
                nc.vector.tensor_tensor(ttv, mb2v, ptb[:, :, 3, :], mn)
                nc.vector.tensor_reduce(
                    out=res_sb[:, g * SUB:(g + 1) * SUB],
                    in_=tt[:].rearrange("p (v k) -> p v k", k=Q),
                    axis=mybir.AxisListType.X, op=mx,
                )
            nc.sync.dma_start(out=res[:, :], in_=res_sb[:])
    nc.compile()
    return nc


# ------------------------------------------------------------ device program

def _build_program(G):
    import concourse.mybir as mybir
    from concourse import bacc
    from concourse.tile import TileContext

    S = G * SPG
    nc = bacc.Bacc(None, target_bir_lowering=False, debug=False)
    tab = nc.declare_dram_parameter("tab", [G, 6, 3072], mybir.dt.bfloat16,
                                    isOutput=False)
    res = nc.declare_dram_parameter("res", [128, S], mybir.dt.float32,
                                    isOutput=True)
    mn = mybir.AluOpType.min
    mx = mybir.AluOpType.max
    f32 = mybir.dt.float32
    bf16 = mybir.dt.bfloat16

    with TileContext(nc) as tc:
        with (
            tc.tile_pool(name="tabs", bufs=4) as tab_pool,
            tc.tile_pool(name="work", bufs=3) as work_pool,
            tc.tile_pool(name="out", bufs=1) as out_pool,
            tc.tile_pool(name="psum", bufs=2, space="PSUM") as psum_pool,
        ):
            res_sb = out_pool.tile([128, S], f32)
            for g in range(G):
                tabt = tab_pool.tile([6, 3072], bf16)
                nc.sync.dma_start(out=tabt[:], in_=tab[g])
                # bf16 hi/lo split-coefficient matmuls: K=6 rows
                # (Xl,Yl,1,Xl,Yl,1) x (Ahi,Bhi,Chi,Alo,Blo,Clo); products are
                # exact (4+8 mantissa bits), accumulated in f32 PSUM.
                pt = psum_pool.tile([128, 2048], f32)
                for s in range(SPG):
                    nc.tensor.matmul(
                        pt[:, s * 256:(s + 1) * 256],
                        lhsT=tabt[:, 2048 + s * 128: 2048 + (s + 1) * 128],
                        rhs=tabt[:, s * 256:(s + 1) * 256],
                        start=True, stop=True,
                    )
                # PSUM per slot: (w1, w0, w2, inv). ScalarE evacuates planes
                # 1..3 in one strided copy; DVE mins w1 (PSUM) against w0;
                # gpsimd handles the w2/inv min; the final min is split
                # between DVE and gpsimd; DVE does the per-slot max-reduce.
                pv = pt[:].rearrange("p (s q k) -> p s q k", q=4, k=SLOT)
                c = work_pool.tile([128, 1536], f32)
                cv = c[:].rearrange("p (q s k) -> p s q k", q=3, k=SLOT)
                nc.scalar.copy(cv[:, :, :, :], pv[:, :, 1:4, :])
                m01 = work_pool.tile([128, 512], f32)
                m23 = work_pool.tile([128, 512], f32)
                tt = work_pool.tile([128, 512], f32)
                m01v = m01[:].rearrange("p (s k) -> p s k", k=SLOT)
                nc.vector.tensor_tensor(
                    m01v, pv[:, :, 0, :],
                    c[:, 0:512].rearrange("p (s k) -> p s k", k=SLOT), mn)
                nc.vector.tensor_tensor(m23[:], c[:, 512:1024],
                                        c[:, 1024:1536], mn)
                nc.vector.tensor_tensor(tt[:], m01[:], m23[:], mn)
                nc.vector.tensor_reduce(
                    out=res_sb[:, g * SPG:(g + 1) * SPG],
                    in_=tt[:].rearrange("p (s k) -> p s k", k=SLOT),
                    axis=mybir.AxisListType.X, op=mx,
                )
            nc.sync.dma_start(out=res[:, :], in_=res_sb[:])
    nc.compile()
    return nc


def _get_program(G):
    if G not in _PROGRAM_CACHE:
        _PROGRAM_CACHE[G] = _build_program(G)
    return _PROGRAM_CACHE[G]


# ------------------------------------------------------------------ epilogue

def _normalize_depth(img):
    """Exactly reference._normalize_depth in numpy f32 (per image batch)."""
    img_inf = (img == FAR).astype(np.float32)
    img_no_back = (F32(1.0) - img_inf) * img
    img_max = img_no_back.max(axis=1, keepdims=True).max(axis=2, keepdims=True)
    img_min = img.min(axis=1, keepdims=True).min(axis=2, keepdims=True)
    nd = (img_max - img) / (img_max - img_min + F32(1e-4))
    return np.clip(nd, 0.0, 1.0).astype(np.float32)


# -------------------------------------------------------------------- kernel

def _get_program2(G, sched, goff, tot_cols):
    key = (G, tot_cols, tuple((s, big, r, ridx is None)
                              for grp in sched for (s, big, r, ridx) in grp))
    if key not in _PROGRAM_CACHE:
        _PROGRAM_CACHE[key] = _build_program2(G, sched, goff, tot_cols)
    return _PROGRAM_CACHE[key]


def kernel(vertices, faces, intr, R, t, dist_coeffs):
    from concourse.bass_utils import run_bass_kernel_spmd

    vertices = np.asarray(vertices, dtype=np.float32)
    faces = np.asarray(faces)
    intr = np.asarray(intr, dtype=np.float32)
    R = np.asarray(R, dtype=np.float32)
    t = np.asarray(t, dtype=np.float32)
    dist_coeffs = np.asarray(dist_coeffs, dtype=np.float32)
    Bn = vertices.shape[0]

    tabs, sched, G, goff, core_positions = _build_plan2(
        vertices, faces, intr, R, t, dist_coeffs)
    tot_cols = tabs[0].shape[1]
    nc = _get_program2(G, sched, goff, tot_cols)
    in_maps = [{"tab": tabs[c]} for c in range(N_CORES)]
    out = run_bass_kernel_spmd(nc, in_maps, list(range(N_CORES)))

    maxinv = np.full((Bn, RAST, RAST), -np.inf, dtype=np.float32)
    for c in range(N_CORES):
        rc = out.results[c]["res"]  # [128, G*SUB]
        for g, grp in enumerate(sched):
            for (s, big, r, ridx) in grp:
                if ridx is None or ridx >= len(core_positions[c]):
                    continue
                b, ty, tx, _idxs = core_positions[c][ridx]
                sub0 = g * SUB + (big * BANKC + r) // Q
                vals = rc[:, sub0:sub0 + s // Q].max(1)
                blk = vals.reshape(TH, TW)
                ysl = slice(ty * TH, (ty + 1) * TH)
                xsl = slice(tx * TW, (tx + 1) * TW)
                maxinv[b, ysl, xsl] = np.maximum(maxinv[b, ysl, xsl], blk)

    covered = maxinv > F32(1e-8)
    zbuf = np.where(covered,
                    F32(1.0) / np.where(covered, maxinv, F32(1.0)),
                    FAR).astype(np.float32)
    dep = _normalize_depth(zbuf)
    mask = covered.astype(np.float32)
    return dep, mask
